# revision 1
# baseline (speedup 1.0000x reference)
"""A3TGCN Trainium2 kernel: 8-core SPMD Bass kernel (self-contained).

Strategy: dense-normalized-adjacency SpMM on the TensorEngine with temporal
batching (cell0 GCN batched over all T upfront; cell1 GCN batched over all T
after the cell0 sweep -- it depends only on h0), one AllGather between the
sweeps, feature-major GRU gates and per-block attention on device.
"""
import sys
import types

sys.path.insert(0, "/opt/trn_rl_repo")

LAST_EXEC_NS = None

N, F, T, H, E, OUT, HEADS = 20000, 32, 12, 64, 320000, 12, 2
NCORES, NPAD = 8, 2560


def _install_profhook():
    try:
        import antenv
    except ImportError:
        return
    if "antenv.axon_hooks" in sys.modules:
        return
    mod = types.ModuleType("antenv.axon_hooks")
    mod._hook = None
    def set_axon_ntff_profile_hook(h):
        mod._hook = h
    def get_axon_ntff_profile_hook():
        return mod._hook
    mod.set_axon_ntff_profile_hook = set_axon_ntff_profile_hook
    mod.get_axon_ntff_profile_hook = get_axon_ntff_profile_hook
    sys.modules["antenv.axon_hooks"] = mod
    antenv.axon_hooks = mod
    try:
        from trn_agent_boot.trn_boot import _ntff_profile_via_ctypes
        set_axon_ntff_profile_hook(
            _ntff_profile_via_ctypes("/opt/axon/libaxon_pjrt.so"))
    except Exception:
        mod._hook = None


import numpy as np
import ml_dtypes

import concourse.bass as bass
import concourse.bacc as bacc
import concourse.mybir as mybir
import concourse.tile as tile

F32 = mybir.dt.float32
BF16 = mybir.dt.bfloat16
FP8 = mybir.dt.float8e4
AX = mybir.AluOpType
AF = mybir.ActivationFunctionType


def build(NPAD, NCORES, T, F, H, OUT, HEADS, NCHUNK=512, SCB=8):
    NG = NPAD * NCORES
    SC = NG // 128
    DC = NPAD // 128
    NSCB = SC // SCB
    W0 = T * F
    W1 = T * H
    G0 = 2 * H + F
    DH = H // HEADS
    NC5 = max(1, NPAD // NCHUNK)
    NCHUNK = NPAD // NC5

    nc = bacc.Bacc("TRN2", target_bir_lowering=False, debug=False,
                   num_devices=NCORES)

    a_in = nc.dram_tensor("a", [DC, SC, 128, 128], BF16, kind="ExternalInput")
    xn_in = nc.dram_tensor("xn", [NG, W0], BF16, kind="ExternalInput")
    xt_in = nc.dram_tensor("xt", [F, T, NPAD], BF16, kind="ExternalInput")
    wg0_in = nc.dram_tensor("wg0", [F, H], BF16, kind="ExternalInput")
    wur0_in = nc.dram_tensor("wur0", [G0, 2 * H], BF16, kind="ExternalInput")
    wc0_in = nc.dram_tensor("wc0", [G0, H], BF16, kind="ExternalInput")
    wg1_in = nc.dram_tensor("wg1", [H, H], BF16, kind="ExternalInput")
    wur1_in = nc.dram_tensor("wur1", [3 * H, 2 * H], BF16, kind="ExternalInput")
    wc1_in = nc.dram_tensor("wc1", [3 * H, H], BF16, kind="ExternalInput")
    wqkv_in = nc.dram_tensor("wqkv", [H, 3 * H], BF16, kind="ExternalInput")
    wop_in = nc.dram_tensor("wop", [H, H], BF16, kind="ExternalInput")
    wout_in = nc.dram_tensor("wout", [H, OUT], BF16, kind="ExternalInput")
    bias_in = nc.dram_tensor("bias", [128, 16], F32, kind="ExternalInput")
    idb_in = nc.dram_tensor("idb", [128, 128], BF16, kind="ExternalInput")
    out_ext = nc.dram_tensor("out", [NPAD, OUT], F32, kind="ExternalOutput")
    ag_out = nc.dram_tensor("ag_shared", [NG, W1], FP8, addr_space="Shared")

    with tile.TileContext(nc) as tc:
        with tc.tile_pool(name="dram", bufs=1, space="DRAM") as dram, \
             tc.tile_pool(name="wsb", bufs=1) as wsb, \
             tc.tile_pool(name="state", bufs=1) as st, \
             tc.tile_pool(name="abuf", bufs=3) as abuf, \
             tc.tile_pool(name="mbuf", bufs=2) as mbuf, \
             tc.tile_pool(name="work", bufs=1) as wk, \
             tc.tile_pool(name="psum", bufs=2, space="PSUM") as pp, \
             tc.tile_pool(name="psumT", bufs=4, space="PSUM") as ppt:

            def load(pool, src, shape, dt):
                t_ = pool.tile(shape, dt, tag=src.name + "_sb")
                nc.sync.dma_start(t_[:], src[:])
                return t_
            wg0 = load(wsb, wg0_in, [F, H], BF16)
            wur0_x = wsb.tile([F, 2 * H], BF16, tag="wur0x")
            nc.sync.dma_start(wur0_x[:], wur0_in[0:F, :])
            wur0_g = wsb.tile([H, 2 * H], BF16, tag="wur0g")
            nc.sync.dma_start(wur0_g[:], wur0_in[F:F + H, :])
            wur0_h = wsb.tile([H, 2 * H], BF16, tag="wur0h")
            nc.sync.dma_start(wur0_h[:], wur0_in[F + H:G0, :])
            wc0_x = wsb.tile([F, H], BF16, tag="wc0x")
            nc.sync.dma_start(wc0_x[:], wc0_in[0:F, :])
            wc0_g = wsb.tile([H, H], BF16, tag="wc0g")
            nc.sync.dma_start(wc0_g[:], wc0_in[F:F + H, :])
            wc0_h = wsb.tile([H, H], BF16, tag="wc0h")
            nc.sync.dma_start(wc0_h[:], wc0_in[F + H:G0, :])
            wg1 = load(wsb, wg1_in, [H, H], BF16)
            wur1_x = wsb.tile([H, 2 * H], BF16, tag="wur1x")
            nc.sync.dma_start(wur1_x[:], wur1_in[0:H, :])
            wur1_g = wsb.tile([H, 2 * H], BF16, tag="wur1g")
            nc.sync.dma_start(wur1_g[:], wur1_in[H:2 * H, :])
            wur1_h = wsb.tile([H, 2 * H], BF16, tag="wur1h")
            nc.sync.dma_start(wur1_h[:], wur1_in[2 * H:3 * H, :])
            wc1_x = wsb.tile([H, H], BF16, tag="wc1x")
            nc.sync.dma_start(wc1_x[:], wc1_in[0:H, :])
            wc1_g = wsb.tile([H, H], BF16, tag="wc1g")
            nc.sync.dma_start(wc1_g[:], wc1_in[H:2 * H, :])
            wc1_h = wsb.tile([H, H], BF16, tag="wc1h")
            nc.sync.dma_start(wc1_h[:], wc1_in[2 * H:3 * H, :])
            wqkv = load(wsb, wqkv_in, [H, 3 * H], BF16)
            wop = load(wsb, wop_in, [H, H], BF16)
            wout = load(wsb, wout_in, [H, OUT], BF16)
            biases = load(wsb, bias_in, [128, 16], F32)
            identb = load(wsb, idb_in, [128, 128], BF16)

            h0T = st.tile([H, NPAD], BF16)
            h1T = st.tile([H, NPAD], BF16)
            g0T = st.tile([H, NPAD], BF16)
            rh = st.tile([H, NPAD], BF16)
            urT = st.tile([2 * H, NPAD], BF16)
            cT = st.tile([H, NPAD], BF16)
            rT = st.tile([H, NPAD], BF16)
            stag = st.tile([128, DC, H], FP8)
            agg_sb = st.tile([128, DC, W1], BF16)
            omT = st.tile([H, NPAD], BF16)
            nc.vector.memset(h0T[:], 0.0)
            nc.vector.memset(h1T[:], 0.0)

            ag_in = dram.tile([NPAD, W1], FP8)
            h0seq_d = dram.tile([T, H, NPAD], BF16)
            h1seq_d = dram.tile([T, H, NPAD], BF16)
            g1_d = dram.tile([T, H, NPAD], BF16)

            def spmm(dram_src, WW, cast=False):
                for scb in range(NSCB):
                    msup = mbuf.tile([128, SCB, W1], BF16, tag="msup")
                    dma = nc.gpsimd.dma_start if cast else nc.sync.dma_start
                    dma(msup[:, :, 0:WW],
                        dram_src[scb * SCB * 128:(scb + 1) * SCB * 128, :]
                        .rearrange("(s p) w -> p s w", p=128))
                    for dc in range(DC):
                        asup = abuf.tile([128, SCB, 128], BF16, tag="asup")
                        nc.sync.dma_start(
                            asup[:], a_in[dc, scb * SCB:(scb + 1) * SCB, :, :]
                            .rearrange("s p d -> p s d"))
                        nw = (WW + 511) // 512
                        for w in range(nw):
                            wlo = w * 512
                            whi = min(WW, wlo + 512)
                            ps = pp.tile([128, 512], F32, tag="spmm")
                            for k in range(SCB):
                                nc.tensor.matmul(
                                    ps[:, :whi - wlo], asup[:, k, :],
                                    msup[:, k, wlo:whi],
                                    start=(k == 0), stop=(k == SCB - 1))
                            if scb == 0:
                                nc.vector.tensor_copy(
                                    agg_sb[:, dc, wlo:whi], ps[:, :whi - wlo])
                            else:
                                nc.vector.tensor_tensor(
                                    agg_sb[:, dc, wlo:whi],
                                    agg_sb[:, dc, wlo:whi], ps[:, :whi - wlo],
                                    op=AX.add)

            # ---------- SpMM-0: agg_sb[:, :, :W0] = A^T @ Xn ----------
            spmm(xn_in, W0)

            # ---------- cell0 sweep ----------
            for t_ in range(T):
                axtb = wk.tile([F, NPAD], BF16, tag="axtb")
                for dc in range(DC):
                    pst = ppt.tile([F, 128], BF16, tag="tr")
                    nc.tensor.matmul(pst[:], agg_sb[:, dc, t_ * F:(t_ + 1) * F],
                                     identb[:], is_transpose=True,
                                     start=True, stop=True)
                    nc.vector.tensor_copy(axtb[:, dc * 128:(dc + 1) * 128], pst[:])
                axt = axtb[:]
                h0 = h0T[:]
                xtb = wk.tile([F, NPAD], BF16, tag="xtb")
                nc.sync.dma_start(xtb[:], xt_in[:, t_, :])
                for ch in range(NC5):
                    sl = slice(ch * NCHUNK, (ch + 1) * NCHUNK)
                    ps = pp.tile([H, NCHUNK], F32, tag="mm")
                    nc.tensor.matmul(ps[:], wg0[:], axt[:, sl],
                                     start=True, stop=True)
                    nc.scalar.activation(g0T[:, sl], ps[:], AF.Sigmoid,
                                         bias=biases[0:H, 2:3])
                for ch in range(NC5):
                    sl = slice(ch * NCHUNK, (ch + 1) * NCHUNK)
                    ps = pp.tile([2 * H, NCHUNK], F32, tag="mm")
                    nc.tensor.matmul(ps[:], wur0_x[:], xtb[:, sl], start=True, stop=False)
                    nc.tensor.matmul(ps[:], wur0_g[:], g0T[:, sl], start=False, stop=False)
                    nc.tensor.matmul(ps[:], wur0_h[:], h0[:, sl], start=False, stop=True)
                    nc.scalar.activation(urT[:, sl], ps[:], AF.Sigmoid,
                                         bias=biases[0:2 * H, 0:1])
                nc.vector.tensor_copy(rT[:], urT[H:2 * H, :])
                nc.vector.tensor_tensor(rh[:], rT[:], h0, op=AX.mult)
                for ch in range(NC5):
                    sl = slice(ch * NCHUNK, (ch + 1) * NCHUNK)
                    ps = pp.tile([H, NCHUNK], F32, tag="mm")
                    nc.tensor.matmul(ps[:], wc0_x[:], xtb[:, sl], start=True, stop=False)
                    nc.tensor.matmul(ps[:], wc0_g[:], g0T[:, sl], start=False, stop=False)
                    nc.tensor.matmul(ps[:], wc0_h[:], rh[:, sl], start=False, stop=True)
                    nc.scalar.activation(cT[:, sl], ps[:], AF.Tanh,
                                         bias=biases[0:H, 1:2])
                nc.vector.tensor_tensor(rh[:], h0, cT[:], op=AX.subtract)
                nc.vector.tensor_tensor(rh[:], rh[:], urT[0:H, :], op=AX.mult)
                nc.vector.tensor_tensor(h0, rh[:], cT[:], op=AX.add)
                nc.sync.dma_start(h0seq_d[t_, :, :], h0)
                # h0w_t = (h0 @ wg1)^T -> node-major -> ag_in[:, t*H:(t+1)*H]
                for ch in range(NC5):
                    sl = slice(ch * NCHUNK, (ch + 1) * NCHUNK)
                    ps = pp.tile([H, NCHUNK], F32, tag="mm")
                    nc.tensor.matmul(ps[:], wg1[:], h0[:, sl], start=True, stop=True)
                    nc.vector.tensor_copy(rh[:, sl], ps[:])
                for dc in range(DC):
                    pst = ppt.tile([128, H], BF16, tag="tr")
                    nc.tensor.matmul(pst[:], rh[:, dc * 128:(dc + 1) * 128],
                                     identb[0:H, 0:H], is_transpose=True,
                                     start=True, stop=True)
                    nc.vector.tensor_copy(stag[:, dc, :], pst[:])
                nc.sync.dma_start(
                    ag_in[:, t_ * H:(t_ + 1) * H]
                    .rearrange("(d p) w -> p d w", p=128), stag[:])

            nc.gpsimd.collective_compute(
                "AllGather", AX.bypass,
                replica_groups=[list(range(NCORES))],
                ins=[ag_in.opt()], outs=[ag_out[:]])

            # ---------- SpMM-1: agg_sb = A^T @ H0W ----------
            spmm(ag_out, W1, cast=True)
            for dc in range(DC):
                g1stag = wk.tile([H, T, 128], BF16, tag="g1stag")
                for t_ in range(T):
                    pst = ppt.tile([H, 128], BF16, tag="tr")
                    nc.tensor.matmul(pst[:], agg_sb[:, dc, t_ * H:(t_ + 1) * H],
                                     identb[:], is_transpose=True,
                                     start=True, stop=True)
                    nc.scalar.activation(g1stag[:, t_, :], pst[:],
                                         AF.Sigmoid, bias=biases[0:H, 5:6])
                nc.sync.dma_start(
                    g1_d[:, :, dc * 128:(dc + 1) * 128]
                    .rearrange("t h n -> h t n"), g1stag[:])

            # ---------- cell1 sweep ----------
            for t_ in range(T):
                h1 = h1T[:]
                g1 = wk.tile([H, NPAD], BF16, tag="g1buf")
                nc.sync.dma_start(g1[:], g1_d[t_, :, :])
                g1 = g1[:]
                h0t = wk.tile([H, NPAD], BF16, tag="h0buf")
                nc.sync.dma_start(h0t[:], h0seq_d[t_, :, :])
                h0t = h0t[:]
                for ch in range(NC5):
                    sl = slice(ch * NCHUNK, (ch + 1) * NCHUNK)
                    ps = pp.tile([2 * H, NCHUNK], F32, tag="mm")
                    nc.tensor.matmul(ps[:], wur1_x[:], h0t[:, sl], start=True, stop=False)
                    nc.tensor.matmul(ps[:], wur1_g[:], g1[:, sl], start=False, stop=False)
                    nc.tensor.matmul(ps[:], wur1_h[:], h1[:, sl], start=False, stop=True)
                    nc.scalar.activation(urT[:, sl], ps[:], AF.Sigmoid,
                                         bias=biases[0:2 * H, 3:4])
                nc.vector.tensor_copy(rT[:], urT[H:2 * H, :])
                nc.vector.tensor_tensor(rh[:], rT[:], h1, op=AX.mult)
                for ch in range(NC5):
                    sl = slice(ch * NCHUNK, (ch + 1) * NCHUNK)
                    ps = pp.tile([H, NCHUNK], F32, tag="mm")
                    nc.tensor.matmul(ps[:], wc1_x[:], h0t[:, sl], start=True, stop=False)
                    nc.tensor.matmul(ps[:], wc1_g[:], g1[:, sl], start=False, stop=False)
                    nc.tensor.matmul(ps[:], wc1_h[:], rh[:, sl], start=False, stop=True)
                    nc.scalar.activation(cT[:, sl], ps[:], AF.Tanh,
                                         bias=biases[0:H, 4:5])
                nc.vector.tensor_tensor(rh[:], h1, cT[:], op=AX.subtract)
                nc.vector.tensor_tensor(rh[:], rh[:], urT[0:H, :], op=AX.mult)
                nc.vector.tensor_tensor(h1, rh[:], cT[:], op=AX.add)
                nc.sync.dma_start(h1seq_d[t_, :, :], h1)

            # ---------- attention (per dst-chunk of 128 nodes) ----------
            for dc in range(DC):
                nsl = slice(dc * 128, (dc + 1) * 128)
                # qkv for this block, all t: moving [64, T*128]
                qkP = wk.tile([2 * H, T, 128], BF16, tag="qkP")
                vP = wk.tile([H, T, 128], BF16, tag="vP")
                h1b_t = wk.tile([H, T, 128], BF16, tag="h1b")
                nc.sync.dma_start(
                    h1b_t[:], h1seq_d[:, :, nsl].rearrange("t h n -> h t n"))
                h1b = h1b_t[:]
                nt = (T * 128 + 511) // 512
                for w in range(nt):
                    lo, hi = w * 512, min(T * 128, (w + 1) * 512)
                    ps = pp.tile([2 * H, 512], F32, tag="mm")
                    nc.tensor.matmul(ps[:, :hi - lo], wqkv[:, 0:2 * H],
                                     h1b.rearrange("p t n -> p (t n)")[:, lo:hi],
                                     start=True, stop=True)
                    nc.scalar.activation(
                        qkP[:].rearrange("p t n -> p (t n)")[:, lo:hi],
                        ps[:, :hi - lo], AF.Identity, bias=biases[0:2 * H, 6:7])
                    ps2 = pp.tile([H, 512], F32, tag="mm")
                    nc.tensor.matmul(ps2[:, :hi - lo], wqkv[:, 2 * H:3 * H],
                                     h1b.rearrange("p t n -> p (t n)")[:, lo:hi],
                                     start=True, stop=True)
                    nc.scalar.activation(
                        vP[:].rearrange("p t n -> p (t n)")[:, lo:hi],
                        ps2[:, :hi - lo], AF.Identity, bias=biases[0:H, 7:8])
                # transpose to node-major
                qN = wk.tile([128, T, H], BF16, tag="qN")
                kN = wk.tile([128, T, H], BF16, tag="kN")
                vv = wk.tile([128, T, H], BF16, tag="vv")
                kPb = wk.tile([H, T, 128], BF16, tag="kPb")
                nc.vector.tensor_copy(kPb[:], qkP[H:2 * H, :, :])
                for t_ in range(T):
                    for src_ap, dst in ((qkP[0:H, t_, :], qN), (kPb[:, t_, :], kN),
                                        (vP[:, t_, :], vv)):
                        pq = ppt.tile([128, H], BF16, tag="tr")
                        nc.tensor.matmul(pq[:], src_ap, identb[0:H, 0:H],
                                         is_transpose=True, start=True, stop=True)
                        nc.vector.tensor_copy(dst[:, t_, :], pq[:])
                prod = wk.tile([128, HEADS, T, T, DH], BF16, tag="prod")
                for hh in range(HEADS):
                    nc.vector.tensor_tensor(
                        prod[:, hh],
                        qN[:, :, hh * DH:(hh + 1) * DH]
                        .unsqueeze(2).broadcast_to([128, T, T, DH]),
                        kN[:, :, hh * DH:(hh + 1) * DH]
                        .unsqueeze(1).broadcast_to([128, T, T, DH]),
                        op=AX.mult)
                sc_t = wk.tile([128, HEADS, T, T], F32, tag="sc")
                nc.vector.tensor_reduce(sc_t[:], prod[:],
                                        axis=mybir.AxisListType.X, op=AX.add)
                esc = wk.tile([128, HEADS, T, T], F32, tag="esc")
                nc.scalar.activation(esc[:], sc_t[:], AF.Exp)
                zsum = wk.tile([128, HEADS, T], F32, tag="z")
                nc.vector.tensor_reduce(zsum[:], esc[:],
                                        axis=mybir.AxisListType.X, op=AX.add)
                zinv = wk.tile([128, HEADS, T], F32, tag="zi")
                nc.vector.reciprocal(zinv[:], zsum[:])
                attn = wk.tile([128, HEADS, T, T], F32, tag="attn")
                nc.vector.tensor_tensor(
                    attn[:], esc[:],
                    zinv[:].unsqueeze(3).broadcast_to([128, HEADS, T, T]),
                    op=AX.mult)
                attn_m = wk.tile([128, HEADS, T], F32, tag="am")
                nc.vector.tensor_reduce(attn_m[:],
                                        attn[:].rearrange("p h t s -> p h s t"),
                                        axis=mybir.AxisListType.X, op=AX.add)
                prod2 = wk.tile([128, HEADS, T, DH], F32, tag="p2")
                for hh in range(HEADS):
                    nc.vector.tensor_tensor(
                        prod2[:, hh, :, :],
                        vv[:, :, hh * DH:(hh + 1) * DH],
                        attn_m[:, hh, :].unsqueeze(2).broadcast_to([128, T, DH]),
                        op=AX.mult)
                om = wk.tile([128, HEADS, DH], F32, tag="om")
                nc.vector.tensor_reduce(om[:],
                                        prod2[:].rearrange("p h t d -> p h d t"),
                                        axis=mybir.AxisListType.X, op=AX.add)
                omb = wk.tile([128, H], BF16, tag="omb")
                nc.vector.tensor_scalar_mul(
                    omb[:], om[:].rearrange("p h d -> p (h d)"), 1.0 / T)
                pot = ppt.tile([H, 128], BF16, tag="tr")
                nc.tensor.matmul(pot[:], omb[:], identb[:], is_transpose=True,
                                 start=True, stop=True)
                nc.vector.tensor_copy(omT[:, nsl], pot[:])

            # out_proj + head
            finT = st.tile([OUT, NPAD], BF16)
            opT = st.tile([H, NPAD], BF16)
            for ch in range(NC5):
                sl = slice(ch * NCHUNK, (ch + 1) * NCHUNK)
                ps = pp.tile([H, NCHUNK], F32, tag="mm")
                nc.tensor.matmul(ps[:], wop[:], omT[:, sl], start=True, stop=True)
                nc.scalar.activation(opT[:, sl], ps[:], AF.Identity,
                                     bias=biases[0:H, 8:9])
                ps2 = pp.tile([OUT, NCHUNK], F32, tag="mm")
                nc.tensor.matmul(ps2[:], wout[:], opT[:, sl], start=True, stop=True)
                nc.scalar.activation(finT[:, sl], ps2[:], AF.Identity,
                                     bias=biases[0:OUT, 9:10])
            fin_nm = st.tile([128, DC, OUT], F32)
            for dc in range(DC):
                pft = ppt.tile([128, OUT], BF16, tag="tr")
                nc.tensor.matmul(pft[:], finT[:, dc * 128:(dc + 1) * 128],
                                 identb[0:OUT, 0:OUT], is_transpose=True,
                                 start=True, stop=True)
                nc.vector.tensor_copy(fin_nm[:, dc, :], pft[:])
            nc.sync.dma_start(out_ext[:].rearrange("(d p) o -> p d o", p=128),
                              fin_nm[:])

    nc.finalize()
    return nc


def prep_inputs(inp, NCORES=8, NPAD=2560):
    N, F, T = np.asarray(inp["x"]).shape
    H = np.asarray(inp["Wg0"]).shape[1]
    OUT = np.asarray(inp["out_w"]).shape[1]
    HEADS = 2
    DH = H // HEADS
    NG = NPAD * NCORES
    W0 = T * F

    src = np.asarray(inp["edge_index"][0])
    dst = np.asarray(inp["edge_index"][1])
    w = np.asarray(inp["edge_attr"])[:, -1].astype(np.float64)

    per = N // NCORES
    old2new = np.zeros(N, np.int64)
    for c in range(NCORES):
        old2new[c * per:(c + 1) * per] = c * NPAD + np.arange(per)
    deg = np.ones(N, np.float64)
    np.add.at(deg, dst, w)
    dinv = 1.0 / np.sqrt(deg)
    A = np.zeros((NG, NG), np.float32)
    coef = (dinv[src] * w * dinv[dst]).astype(np.float32)
    np.add.at(A, (old2new[src], old2new[dst]), coef)
    A[old2new, old2new] += (1.0 / deg).astype(np.float32)
    A = A.astype(ml_dtypes.bfloat16)

    x = np.asarray(inp["x"], np.float32)
    xn = np.zeros((NG, W0), np.float32)
    xn[old2new, :] = x.transpose(0, 2, 1).reshape(N, W0)
    xn = xn.astype(ml_dtypes.bfloat16)

    def bf(a):
        return np.ascontiguousarray(np.asarray(a, np.float32)).astype(ml_dtypes.bfloat16)

    ipw = np.asarray(inp["in_proj_w"], np.float32)
    ipb = np.asarray(inp["in_proj_b"], np.float32)
    s = 1.0 / np.sqrt(DH)
    wqkv = np.concatenate([ipw[0:H].T * s, ipw[H:2 * H].T, ipw[2 * H:].T], axis=1)

    bias = np.zeros((128, 16), np.float32)
    bias[0:H, 0] = np.asarray(inp["bu0"]); bias[H:2 * H, 0] = np.asarray(inp["br0"])
    bias[0:H, 1] = np.asarray(inp["bc0"])
    bias[0:H, 2] = np.asarray(inp["bg0"])
    bias[0:H, 3] = np.asarray(inp["bu1"]); bias[H:2 * H, 3] = np.asarray(inp["br1"])
    bias[0:H, 4] = np.asarray(inp["bc1"])
    bias[0:H, 5] = np.asarray(inp["bg1"])
    bias[0:H, 6] = ipb[0:H] * s; bias[H:2 * H, 6] = ipb[H:2 * H]
    bias[0:H, 7] = ipb[2 * H:]
    bias[0:H, 8] = np.asarray(inp["out_proj_b"])
    bias[0:OUT, 9] = np.asarray(inp["out_b"])

    wur0 = np.concatenate([np.asarray(inp["Wu0"]), np.asarray(inp["Wr0"])], axis=1)
    wur1 = np.concatenate([np.asarray(inp["Wu1"]), np.asarray(inp["Wr1"])], axis=1)
    idb = np.eye(128, dtype=np.float32).astype(ml_dtypes.bfloat16)

    DCn = NPAD // 128
    SCn = NG // 128
    in_maps = []
    for c in range(NCORES):
        Ac = A[:, c * NPAD:(c + 1) * NPAD]
        Ac = np.ascontiguousarray(
            Ac.reshape(SCn, 128, DCn, 128).transpose(2, 0, 1, 3))
        xtc = np.zeros((F, T, NPAD), np.float32)
        xtc[:, :, 0:per] = x[c * per:(c + 1) * per].transpose(1, 2, 0)
        in_maps.append(dict(
            a=Ac, xn=xn, xt=xtc.astype(ml_dtypes.bfloat16),
            wg0=bf(inp["Wg0"]), wur0=bf(wur0), wc0=bf(inp["Wc0"]),
            wg1=bf(inp["Wg1"]), wur1=bf(wur1), wc1=bf(inp["Wc1"]),
            wqkv=bf(wqkv), wop=bf(np.asarray(inp["out_proj_w"], np.float32).T),
            wout=bf(inp["out_w"]), bias=bias, idb=idb,
        ))
    return in_maps


def assemble_output(results, N, NCORES=8, NPAD=2560, OUT=12):
    per = N // NCORES
    out = np.zeros((N, OUT), np.float32)
    for c in range(NCORES):
        out[c * per:(c + 1) * per] = results[c]["out"][0:per]
    return out


_NC_CACHE = {}


def _get_nc():
    if "nc" not in _NC_CACHE:
        _NC_CACHE["nc"] = build(NPAD, NCORES, T, F, H, OUT, HEADS)
    return _NC_CACHE["nc"]


def kernel(**inputs):
    global LAST_EXEC_NS
    _install_profhook()
    from concourse.bass_utils import run_bass_kernel_spmd
    nc = _get_nc()
    in_maps = prep_inputs(inputs, NCORES=NCORES, NPAD=NPAD)
    try:
        res = run_bass_kernel_spmd(nc, in_maps, list(range(NCORES)), trace=True)
    except Exception:
        res = run_bass_kernel_spmd(nc, in_maps, list(range(NCORES)), trace=False)
    LAST_EXEC_NS = res.exec_time_ns
    return assemble_output(res.results, N, NCORES=NCORES, NPAD=NPAD, OUT=OUT)



# revision 17
# speedup vs baseline: 2.3794x; 2.3794x over previous
"""A3TGCN Trainium2 kernel: 8-core SPMD Bass kernel (self-contained).

Strategy (v2, sparse): exploit graph sparsity instead of a dense adjacency.
Edges are partitioned by destination core and grouped into per-dst-chunk
(128 nodes) blocks of 128 edges. Each SpMM becomes gather + compact
scatter-matmul: out[128dst, W] = sum_b S_b^T @ M_b with S_b [128edge,128dst]
holding the GCN norm coefficients. SpMM-0 messages (raw x rows by edge
source) are gathered on the host; SpMM-1 messages (h0 rows from the fp8
AllGather buffer) are gathered on device via dma_gather. Wg1 is applied
after aggregation (A^T (h0 Wg1) == (A^T h0) Wg1) so the collective carries
raw h0. GRU gates run feature-major with stacked-contraction matmuls; the
out_proj+head matrices are fused on the host; QKV bias is folded in as a
ones-row of the stationary operand.
"""
import sys
import types

sys.path.insert(0, "/opt/trn_rl_repo")

LAST_EXEC_NS = None

N, F, T, H, E, OUT, HEADS = 20000, 32, 12, 64, 320000, 12, 2
NCORES, PER, NPAD = 8, 2500, 2560
NG = NPAD * NCORES
DC = NPAD // 128
W0, W1 = T * F, T * H
DH = H // HEADS
GRP = 4
CPG = DC // GRP           # chunks per group
NW = NPAD // GRP          # nodes per group


def _install_profhook():
    try:
        import antenv
    except ImportError:
        return
    if "antenv.axon_hooks" in sys.modules:
        return
    mod = types.ModuleType("antenv.axon_hooks")
    mod._hook = None
    def set_axon_ntff_profile_hook(h):
        mod._hook = h
    def get_axon_ntff_profile_hook():
        return mod._hook
    mod.set_axon_ntff_profile_hook = set_axon_ntff_profile_hook
    mod.get_axon_ntff_profile_hook = get_axon_ntff_profile_hook
    sys.modules["antenv.axon_hooks"] = mod
    antenv.axon_hooks = mod
    try:
        from trn_agent_boot.trn_boot import _ntff_profile_via_ctypes
        set_axon_ntff_profile_hook(
            _ntff_profile_via_ctypes("/opt/axon/libaxon_pjrt.so"))
    except Exception:
        mod._hook = None


import numpy as np
import ml_dtypes

import concourse.bass as bass
import concourse.bacc as bacc
import concourse.mybir as mybir
import concourse.tile as tile

F32 = mybir.dt.float32
BF16 = mybir.dt.bfloat16
FP8 = mybir.dt.float8e4
I16 = mybir.dt.int16
AX = mybir.AluOpType
AF = mybir.ActivationFunctionType


def build(NB):
    NE = NB * 128             # padded edges per dst chunk
    IDXC = NE // 16           # idx columns per chunk

    nc = bacc.Bacc("TRN2", target_bir_lowering=False, debug=False,
                   num_devices=NCORES)

    s_in = nc.dram_tensor("s", [DC, 128, NB * 128], BF16, kind="ExternalInput")
    xg_in = nc.dram_tensor("xg", [DC, 128, NB * W0], FP8, kind="ExternalInput")
    idx_in = nc.dram_tensor("idx", [128, DC * IDXC], I16, kind="ExternalInput")
    xt_in = nc.dram_tensor("xt", [F, T, NPAD], BF16, kind="ExternalInput")
    wg0_in = nc.dram_tensor("wg0", [128, 4 * H], BF16, kind="ExternalInput")
    wg1_in = nc.dram_tensor("wg1", [128, 2 * H], BF16, kind="ExternalInput")
    wur0xg_in = nc.dram_tensor("wur0xg", [F + H, 2 * H], BF16, kind="ExternalInput")
    wur0h_in = nc.dram_tensor("wur0h", [H, 2 * H], BF16, kind="ExternalInput")
    wc0xg_in = nc.dram_tensor("wc0xg", [F + H, H], BF16, kind="ExternalInput")
    wc0h_in = nc.dram_tensor("wc0h", [H, H], BF16, kind="ExternalInput")
    wur1xg_in = nc.dram_tensor("wur1xg", [2 * H, 2 * H], BF16, kind="ExternalInput")
    wur1h_in = nc.dram_tensor("wur1h", [H, 2 * H], BF16, kind="ExternalInput")
    wc1xg_in = nc.dram_tensor("wc1xg", [2 * H, H], BF16, kind="ExternalInput")
    wc1h_in = nc.dram_tensor("wc1h", [H, H], BF16, kind="ExternalInput")
    wqkv_in = nc.dram_tensor("wqkv", [H + 1, 3 * H], BF16, kind="ExternalInput")
    wf_in = nc.dram_tensor("wf", [H + 1, OUT], BF16, kind="ExternalInput")
    bias_in = nc.dram_tensor("bias", [128, 16], F32, kind="ExternalInput")
    idb_in = nc.dram_tensor("idb", [128, 128], BF16, kind="ExternalInput")
    out_ext = nc.dram_tensor("out", [NPAD, OUT], F32, kind="ExternalOutput")
    ag_out = nc.dram_tensor("ag_shared", [NG, W1], FP8, addr_space="Shared")

    with tile.TileContext(nc) as tc:
        with tc.tile_pool(name="dram", bufs=1, space="DRAM") as dram, \
             tc.tile_pool(name="wsb", bufs=1) as wsb, \
             tc.tile_pool(name="state", bufs=1) as st, \
             tc.tile_pool(name="spool", bufs=3) as spool, \
             tc.tile_pool(name="mpool", bufs=3) as mpool, \
             tc.tile_pool(name="work", bufs=2) as wk, \
             tc.tile_pool(name="rb", bufs=2) as rb, \
             tc.tile_pool(name="qkvp", bufs=2) as qkvp, \
             tc.tile_pool(name="attn", bufs=2) as at, \
             tc.tile_pool(name="pmm", bufs=4, space="PSUM") as pmm, \
             tc.tile_pool(name="pmmc", bufs=2, space="PSUM") as pmmc, \
             tc.tile_pool(name="ptr", bufs=2, space="PSUM") as ptr:

            def load(pool, src, shape, dt, tag=None):
                t_ = pool.tile(shape, dt, tag=tag or (src.name + "_sb"))
                nc.sync.dma_start(t_[:], src[:])
                return t_

            wg0p = load(wsb, wg0_in, [128, 4, H], BF16)       # 4 shifted copies
            wg1p = load(wsb, wg1_in, [128, 2, H], BF16)       # 2 shifted copies
            wur0xg = load(wsb, wur0xg_in, [F + H, 2 * H], BF16)
            wur0h = load(wsb, wur0h_in, [H, 2 * H], BF16)
            wc0xg = load(wsb, wc0xg_in, [F + H, H], BF16)
            wc0h = load(wsb, wc0h_in, [H, H], BF16)
            wur1xg = load(wsb, wur1xg_in, [2 * H, 2 * H], BF16)
            wur1h = load(wsb, wur1h_in, [H, 2 * H], BF16)
            wc1xg = load(wsb, wc1xg_in, [2 * H, H], BF16)
            wc1h = load(wsb, wc1h_in, [H, H], BF16)
            wqkv = load(wsb, wqkv_in, [H + 1, 3 * H], BF16)
            wf = load(wsb, wf_in, [H + 1, OUT], BF16)
            biases = load(wsb, bias_in, [128, 16], F32)
            identb = load(wsb, idb_in, [128, 128], BF16)
            idxs = load(wsb, idx_in, [128, DC * IDXC], I16)

            h0T = st.tile([H, NPAD], BF16)
            h1T = st.tile([H, NPAD], BF16)
            urT = st.tile([2 * H, NPAD], BF16)
            rT = st.tile([H, NPAD], BF16)
            rh = st.tile([H, NPAD], BF16)
            cT = st.tile([H, NPAD], BF16)
            fin_nm = st.tile([128, DC, OUT], F32)
            nc.vector.memset(h0T[:], 0.0)
            nc.vector.memset(h1T[:], 0.0)

            stag = st.tile([128, DC, H], FP8)
            h0seq_d = dram.tile([T, H, NPAD], BF16)
            h1seq_d = dram.tile([T, H, NPAD], BF16)
            g0_d = dram.tile([T, H, NPAD], BF16)
            g1_d = dram.tile([T, H, NPAD], BF16)
            ag_in = dram.tile([NPAD, W1], FP8)

            def spmm_post(d, a_nm, nsq, wp, nvar, bcol, g_d):
                # a_nm [128, nsq*128] bf16 node-major -> feature-major squares,
                # then per-t W matmul + sigmoid -> g_d[:, :, d*128:(d+1)*128]
                af = wk.tile([128, 6, 128], BF16, tag="af")
                for s in range(nsq):
                    pst = ptr.tile([128, 128], BF16, tag="tr")
                    nc.tensor.matmul(pst[:], a_nm[:, s * 128:(s + 1) * 128],
                                     identb[:], is_transpose=True,
                                     start=True, stop=True)
                    nc.vector.tensor_copy(af[:, s, :], pst[:])
                gblk = wk.tile([H, T, 128], BF16, tag="gblk")
                for half in range(3):
                    psg = pmmc.tile([H, 4 * 128], F32, tag="mmc")
                    for j in range(4):
                        t_ = half * 4 + j
                        nc.tensor.matmul(psg[:, j * 128:(j + 1) * 128],
                                         wp[:, t_ % nvar, :],
                                         af[:, t_ // nvar, :],
                                         start=True, stop=True)
                    nc.scalar.activation(
                        gblk[:, half * 4:(half + 1) * 4, :]
                        .rearrange("h t n -> h (t n)"), psg[:],
                        AF.Sigmoid, bias=biases[0:H, bcol:bcol + 1])
                nc.sync.dma_start(
                    g_d[:, :, d * 128:(d + 1) * 128].rearrange("t h n -> h t n"),
                    gblk[:])

            # ---------- P1: SpMM-0 (host-gathered messages) + g0 ----------
            for d in range(DC):
                s_sb = spool.tile([128, NB, 128], BF16, tag="s")
                nc.sync.dma_start(s_sb[:].rearrange("p b j -> p (b j)"), s_in[d])
                xg_sb = mpool.tile([128, NB, W0], FP8, tag="msg0")
                nc.sync.dma_start(xg_sb[:].rearrange("p b j -> p (b j)"), xg_in[d])
                ps0 = pmm.tile([128, 512], F32, tag="mm")
                for b in range(NB):
                    nc.tensor.matmul(ps0[:, 0:W0], s_sb[:, b, :], xg_sb[:, b, :],
                                     start=(b == 0), stop=(b == NB - 1))
                a0 = wk.tile([128, W1], BF16, tag="anm")
                nc.scalar.activation(a0[:, 0:W0], ps0[:, 0:W0], AF.Identity)
                spmm_post(d, a0, 3, wg0p, 4, 2, g0_d)

            # ---------- P2: cell0 sweep (full width) ----------
            for t_ in range(T):
                xg0 = rb.tile([128, NPAD], BF16, tag="xg")
                nc.sync.dma_start(xg0[0:F, :], xt_in[:, t_, :])
                nc.sync.dma_start(xg0[F:F + H, :], g0_d[t_])
                for ch in range(5):
                    sl = slice(ch * 512, (ch + 1) * 512)
                    ps = pmm.tile([128, 512], F32, tag="mm")
                    nc.tensor.matmul(ps[:], wur0xg[:], xg0[0:F + H, sl],
                                     start=True, stop=False)
                    nc.tensor.matmul(ps[:], wur0h[:], h0T[:, sl],
                                     start=False, stop=True)
                    nc.scalar.activation(urT[:, sl], ps[:], AF.Sigmoid,
                                         bias=biases[0:2 * H, 0:1])
                    nc.vector.tensor_copy(rT[:, sl], urT[H:2 * H, sl])
                    nc.vector.tensor_tensor(rh[:, sl], rT[:, sl],
                                            h0T[:, sl], op=AX.mult)
                    psc = pmmc.tile([H, 512], F32, tag="mmc")
                    nc.tensor.matmul(psc[:], wc0xg[:], xg0[0:F + H, sl],
                                     start=True, stop=False)
                    nc.tensor.matmul(psc[:], wc0h[:], rh[:, sl],
                                     start=False, stop=True)
                    nc.scalar.activation(cT[:, sl], psc[:], AF.Tanh,
                                         bias=biases[0:H, 1:2])
                    nc.vector.tensor_tensor(rh[:, sl], h0T[:, sl], cT[:, sl],
                                            op=AX.subtract)
                    nc.vector.tensor_tensor(rh[:, sl], rh[:, sl], urT[0:H, sl],
                                            op=AX.mult)
                    nc.vector.tensor_tensor(h0T[:, sl], rh[:, sl], cT[:, sl],
                                            op=AX.add)
                nc.sync.dma_start(h0seq_d[t_], h0T[:])
                # stage node-major fp8 rows of h0_t into ag_in
                for d in range(DC):
                    pst = ptr.tile([128, H], BF16, tag="tr")
                    nc.tensor.matmul(pst[:], h0T[:, d * 128:(d + 1) * 128],
                                     identb[0:H, 0:H], is_transpose=True,
                                     start=True, stop=True)
                    nc.vector.tensor_copy(stag[:, d, :], pst[:])
                nc.sync.dma_start(
                    ag_in[:, t_ * H:(t_ + 1) * H]
                    .rearrange("(d p) w -> p d w", p=128), stag[:])

            # ---------- P3: AllGather ----------
            nc.gpsimd.collective_compute(
                "AllGather", AX.bypass,
                replica_groups=[list(range(NCORES))],
                ins=[ag_in.opt()], outs=[ag_out[:]])

            # ---------- P4/P5/P6 pipelined per node-group ----------
            for g in range(GRP):
                # P4: SpMM-1 + g1 for this group's chunks
                for dd in range(CPG):
                    d = g * CPG + dd
                    g_sb = mpool.tile([128, NB, W1], FP8, tag="msg")
                    import os
                    if os.environ.get("NOGATHER"):
                        nc.sync.dma_start(
                            g_sb[:].rearrange("p b w -> p (b w)"),
                            ag_out[0:NB * 128, :]
                            .rearrange("(p b) w -> p (b w)", p=128))
                    else:
                        nc.gpsimd.dma_gather(
                            g_sb[:], ag_out[:],
                            idxs[:, d * IDXC:(d + 1) * IDXC],
                            NE, NE, W1)
                    s_sb = spool.tile([128, NB, 128], BF16, tag="s")
                    nc.sync.dma_start(s_sb[:].rearrange("p b j -> p (b j)"),
                                      s_in[d])
                    a1 = wk.tile([128, W1], BF16, tag="anm")
                    for half in range(2):
                        ps1 = pmm.tile([128, 512], F32, tag="mm")
                        lo = half * 384
                        for b in range(NB):
                            nc.tensor.matmul(ps1[:, 0:384], s_sb[:, b, :],
                                             g_sb[:, b, lo:lo + 384],
                                             start=(b == 0), stop=(b == NB - 1))
                        nc.scalar.activation(a1[:, lo:lo + 384], ps1[:, 0:384],
                                             AF.Identity)
                    spmm_post(d, a1, 6, wg1p, 2, 5, g1_d)

                # P5: cell1 sweep for this node group
                gsl = slice(g * NW, (g + 1) * NW)
                for t_ in range(T):
                    xg1 = rb.tile([128, NPAD], BF16, tag="xg1")
                    nc.sync.dma_start(xg1[0:H, gsl], h0seq_d[t_, :, gsl])
                    nc.sync.dma_start(xg1[H:2 * H, gsl], g1_d[t_, :, gsl])
                    for ch in range(NW // 320):
                        sl = slice(g * NW + ch * 320, g * NW + (ch + 1) * 320)
                        ps = pmm.tile([128, 512], F32, tag="mm")
                        nc.tensor.matmul(ps[:, 0:320], wur1xg[:], xg1[:, sl],
                                         start=True, stop=False)
                        nc.tensor.matmul(ps[:, 0:320], wur1h[:], h1T[:, sl],
                                         start=False, stop=True)
                        nc.scalar.activation(urT[:, sl], ps[:, 0:320], AF.Sigmoid,
                                             bias=biases[0:2 * H, 3:4])
                        nc.vector.tensor_copy(rT[:, sl], urT[H:2 * H, sl])
                        nc.vector.tensor_tensor(rh[:, sl], rT[:, sl],
                                                h1T[:, sl], op=AX.mult)
                        psc = pmmc.tile([H, 512], F32, tag="mmc")
                        nc.tensor.matmul(psc[:, 0:320], wc1xg[:], xg1[:, sl],
                                         start=True, stop=False)
                        nc.tensor.matmul(psc[:, 0:320], wc1h[:], rh[:, sl],
                                         start=False, stop=True)
                        nc.scalar.activation(cT[:, sl], psc[:, 0:320], AF.Tanh,
                                             bias=biases[0:H, 4:5])
                        nc.vector.tensor_tensor(rh[:, sl], h1T[:, sl], cT[:, sl],
                                                op=AX.subtract)
                        nc.vector.tensor_tensor(rh[:, sl], rh[:, sl], urT[0:H, sl],
                                                op=AX.mult)
                        nc.vector.tensor_tensor(h1T[:, sl], rh[:, sl], cT[:, sl],
                                                op=AX.add)
                    nc.sync.dma_start(h1seq_d[t_, :, gsl], h1T[:, gsl])

                # P6: QKV + attention + head for this group's chunks
                for dd in range(CPG):
                    d = g * CPG + dd
                    h1rb = rb.tile([H + 1, T, 128], BF16, tag="h1rb")
                    nc.vector.memset(h1rb[H:H + 1, :, :], 1.0)
                    nc.sync.dma_start(
                        h1rb[0:H, :, :],
                        h1seq_d[:, :, d * 128:(d + 1) * 128]
                        .rearrange("t h n -> h t n"))
                    qkv = qkvp.tile([128, T, 3 * H], BF16, tag="qkv")
                    for tp in range(T // 2):
                        psq = pmm.tile([128, 2, 3 * H], F32, tag="mm")
                        for j in range(2):
                            nc.tensor.matmul(psq[:, j, :], h1rb[:, tp * 2 + j, :],
                                             wqkv[:], start=True, stop=True)
                        nc.scalar.activation(
                            qkv[:, tp * 2:tp * 2 + 2, :]
                            .rearrange("p t k -> p (t k)"),
                            psq[:].rearrange("p t k -> p (t k)"), AF.Identity)
                    om = at.tile([128, H], F32, tag="om")
                    for hh in range(HEADS):
                        qs = qkv[:, :, hh * DH:(hh + 1) * DH]
                        ks = qkv[:, :, H + hh * DH:H + (hh + 1) * DH]
                        vs = qkv[:, :, 2 * H + hh * DH:2 * H + (hh + 1) * DH]
                        prod = at.tile([128, T, T, DH], BF16, tag="prod")
                        nc.vector.tensor_tensor(
                            prod[:],
                            qs.unsqueeze(2).broadcast_to([128, T, T, DH]),
                            ks.unsqueeze(1).broadcast_to([128, T, T, DH]),
                            op=AX.mult)
                        sc = at.tile([128, T, T], F32, tag="sc")
                        nc.vector.tensor_reduce(sc[:], prod[:],
                                                axis=mybir.AxisListType.X,
                                                op=AX.add)
                        esc = at.tile([128, T, T], F32, tag="esc")
                        nc.scalar.activation(
                            esc[:].rearrange("p t s -> p (t s)"),
                            sc[:].rearrange("p t s -> p (t s)"), AF.Exp)
                        z = at.tile([128, T], F32, tag="z")
                        nc.vector.tensor_reduce(z[:], esc[:],
                                                axis=mybir.AxisListType.X,
                                                op=AX.add)
                        zi = at.tile([128, T], F32, tag="zi")
                        nc.vector.reciprocal(zi[:], z[:])
                        wsc = at.tile([128, T, T], BF16, tag="wsc")
                        nc.vector.tensor_tensor(
                            wsc[:], esc[:],
                            zi[:].unsqueeze(2).broadcast_to([128, T, T]),
                            op=AX.mult)
                        am = at.tile([128, T], F32, tag="am")
                        nc.vector.tensor_reduce(
                            am[:], wsc[:].rearrange("p t s -> p s t"),
                            axis=mybir.AxisListType.X, op=AX.add)
                        p2 = at.tile([128, T, DH], F32, tag="p2")
                        nc.vector.tensor_tensor(
                            p2[:], vs,
                            am[:].unsqueeze(2).broadcast_to([128, T, DH]),
                            op=AX.mult)
                        nc.vector.tensor_reduce(
                            om[:, hh * DH:(hh + 1) * DH],
                            p2[:].rearrange("p s d -> p d s"),
                            axis=mybir.AxisListType.X, op=AX.add)
                    om_bf = at.tile([128, H], BF16, tag="ombf")
                    nc.vector.tensor_copy(om_bf[:], om[:])
                    pso = ptr.tile([H, 128], BF16, tag="tr")
                    nc.tensor.matmul(pso[:], om_bf[:], identb[:],
                                     is_transpose=True, start=True, stop=True)
                    omf = wk.tile([H + 1, 128], BF16, tag="omf")
                    nc.vector.memset(omf[H:H + 1, :], 1.0)
                    nc.vector.tensor_copy(omf[0:H, :], pso[:])
                    psf = pmm.tile([128, OUT], F32, tag="mm")
                    nc.tensor.matmul(psf[:], omf[:], wf[:],
                                     start=True, stop=True)
                    nc.vector.tensor_copy(fin_nm[:, d, :], psf[:])

            nc.sync.dma_start(out_ext[:].rearrange("(d p) o -> p d o", p=128),
                              fin_nm[:])

    nc.finalize()
    return nc


def prep_inputs(inp):
    x = np.asarray(inp["x"], np.float32)
    src = np.asarray(inp["edge_index"][0]).astype(np.int64)
    dst = np.asarray(inp["edge_index"][1]).astype(np.int64)
    w = np.asarray(inp["edge_attr"])[:, -1].astype(np.float64)

    deg = np.ones(N, np.float64)
    np.add.at(deg, dst, w)
    dinv = 1.0 / np.sqrt(deg)

    src_all = np.concatenate([src, np.arange(N)])
    dst_all = np.concatenate([dst, np.arange(N)])
    coef_all = np.concatenate([dinv[src] * w * dinv[dst], 1.0 / deg]).astype(np.float32)
    srcrow_all = (src_all // PER) * NPAD + (src_all % PER)

    core_of = dst_all // PER
    lc = dst_all - core_of * PER
    chunk = lc // 128
    counts = np.zeros((NCORES, DC), np.int64)
    for c in range(NCORES):
        counts[c] = np.bincount(chunk[core_of == c], minlength=DC)
    NB = int(np.ceil(counts.max() / 128))
    NE = NB * 128
    IDXC = NE // 16

    x_nm = np.ascontiguousarray(x.transpose(0, 2, 1).reshape(N, W0))

    def bf(a):
        return np.ascontiguousarray(np.asarray(a, np.float32)).astype(ml_dtypes.bfloat16)

    def f8(a):
        return np.ascontiguousarray(np.asarray(a, np.float32)).astype(ml_dtypes.float8_e4m3)

    # shared weights
    Wu0, Wr0, Wc0 = [np.asarray(inp[k], np.float32) for k in ("Wu0", "Wr0", "Wc0")]
    Wu1, Wr1, Wc1 = [np.asarray(inp[k], np.float32) for k in ("Wu1", "Wr1", "Wc1")]
    wur0 = np.concatenate([Wu0, Wr0], 1)
    wur1 = np.concatenate([Wu1, Wr1], 1)
    wg0p = np.zeros((128, 4, H), np.float32)
    for v in range(4):
        wg0p[v * F:(v + 1) * F, v] = np.asarray(inp["Wg0"], np.float32)
    wg1p = np.zeros((128, 2, H), np.float32)
    for v in range(2):
        wg1p[v * H:(v + 1) * H, v] = np.asarray(inp["Wg1"], np.float32)

    ipw = np.asarray(inp["in_proj_w"], np.float32)
    ipb = np.asarray(inp["in_proj_b"], np.float32)
    s = 1.0 / np.sqrt(DH)
    wqkv = np.zeros((H + 1, 3 * H), np.float32)
    wqkv[0:H] = np.concatenate([ipw[0:H].T * s, ipw[H:2 * H].T, ipw[2 * H:].T], 1)
    wqkv[H] = np.concatenate([ipb[0:H] * s, ipb[H:2 * H], ipb[2 * H:]])
    wf = np.zeros((H + 1, OUT), np.float32)
    wf[0:H] = (np.asarray(inp["out_proj_w"], np.float32).T
               @ np.asarray(inp["out_w"], np.float32)) / T
    wf[H] = (np.asarray(inp["out_proj_b"], np.float32)
             @ np.asarray(inp["out_w"], np.float32) + np.asarray(inp["out_b"]))

    bias = np.zeros((128, 16), np.float32)
    bias[0:H, 0] = np.asarray(inp["bu0"]); bias[H:2 * H, 0] = np.asarray(inp["br0"])
    bias[0:H, 1] = np.asarray(inp["bc0"])
    bias[0:H, 2] = np.asarray(inp["bg0"])
    bias[0:H, 3] = np.asarray(inp["bu1"]); bias[H:2 * H, 3] = np.asarray(inp["br1"])
    bias[0:H, 4] = np.asarray(inp["bc1"])
    bias[0:H, 5] = np.asarray(inp["bg1"])
    idb = np.eye(128, dtype=np.float32)

    shared = dict(
        xt=None, wg0=bf(wg0p.reshape(128, 4 * H)), wg1=bf(wg1p.reshape(128, 2 * H)),
        wur0xg=bf(wur0[0:F + H]), wur0h=bf(wur0[F + H:]),
        wc0xg=bf(Wc0[0:F + H]), wc0h=bf(Wc0[F + H:]),
        wur1xg=bf(wur1[0:2 * H]), wur1h=bf(wur1[2 * H:]),
        wc1xg=bf(Wc1[0:2 * H]), wc1h=bf(Wc1[2 * H:]),
        wqkv=bf(wqkv), wf=bf(wf), bias=bias, idb=bf(idb),
    )

    in_maps = []
    for c in range(NCORES):
        m = core_of == c
        s_r, d_l, cf, ch = srcrow_all[m], lc[m], coef_all[m], chunk[m]
        order = np.argsort(ch, kind="stable")
        s_r, d_l, cf, ch = s_r[order], d_l[order], cf[order], ch[order]
        S = np.zeros((DC, NB, 128, 128), np.float32)
        idxv = np.zeros((DC, NE), np.int64)
        Xg = np.zeros((DC, NE, W0), np.float32)
        pos = 0
        for d in range(DC):
            cnt = counts[c, d]
            sl = slice(pos, pos + cnt)
            e = np.arange(cnt)
            S[d, e // 128, e % 128, d_l[sl] % 128] = cf[sl]
            idxv[d, :cnt] = s_r[sl]
            Xg[d, :cnt] = x_nm[(s_r[sl] // NPAD) * PER + (s_r[sl] % NPAD)]
            pos += cnt
        # device layouts
        S_dev = S.transpose(0, 2, 1, 3).reshape(DC, 128, NB * 128)
        Xg_dev = Xg.reshape(DC, NB, 128, W0).transpose(0, 2, 1, 3).reshape(
            DC, 128, NB * W0)
        idx16 = np.zeros((16, DC * IDXC), np.int16)
        for d in range(DC):
            v = idxv[d].reshape(IDXC, 16).T.astype(np.int16)
            idx16[:, d * IDXC:(d + 1) * IDXC] = v
        idx_dev = np.tile(idx16, (8, 1))

        xtc = np.zeros((F, T, NPAD), np.float32)
        xtc[:, :, 0:PER] = x[c * PER:(c + 1) * PER].transpose(1, 2, 0)

        im = dict(shared)
        im["s"] = bf(S_dev)
        im["xg"] = f8(Xg_dev)
        im["idx"] = np.ascontiguousarray(idx_dev)
        im["xt"] = bf(xtc)
        in_maps.append(im)
    return in_maps, NB


def assemble_output(results):
    out = np.zeros((N, OUT), np.float32)
    for c in range(NCORES):
        out[c * PER:(c + 1) * PER] = results[c]["out"][0:PER]
    return out


_NC_CACHE = {}


def kernel(**inputs):
    global LAST_EXEC_NS
    _install_profhook()
    from concourse.bass_utils import run_bass_kernel_spmd
    in_maps, NB = prep_inputs(inputs)
    if NB not in _NC_CACHE:
        _NC_CACHE[NB] = build(NB)
    nc = _NC_CACHE[NB]
    try:
        res = run_bass_kernel_spmd(nc, in_maps, list(range(NCORES)), trace=True)
    except Exception:
        res = run_bass_kernel_spmd(nc, in_maps, list(range(NCORES)), trace=False)
    LAST_EXEC_NS = res.exec_time_ns
    return assemble_output(res.results)
